# revision 5
# baseline (speedup 1.0000x reference)
"""Expert-parallel MoE GEGLU MLP (RMSNorm -> c_fc -> GEGLU -> c_proj) on 8
Trainium2 NeuronCores.

Sharding: expert-parallel. Core e computes the full MLP for expert e's tokens
(x[:, e] -> [8192, 768]); no collectives. gamma*sqrt(D) is folded into c_fc
and mult_bias into c_proj on the host, so the device kernel computes:

    h   = x / ||x||_2            (per token, fp32 accumulate)
    u   = h @ W1                 (bf16 x bf16 -> fp32 PSUM)
    g   = gelu(u_gate) * u_val   (exact erf gelu on ACT)
    out = g @ W2                 (bf16 x bf16 -> fp32 PSUM)

Layout: tokens stream in blocks of 512. The norm runs with tokens on
partitions; h is then PE-transposed so both GEMMs run with the contraction
dim on partitions (hidden resp. d on PSUM partitions), and the final output
is PE-transposed back to token-major before the store.
"""

from contextlib import ExitStack

import ml_dtypes
import numpy as np

import concourse.bass as bass
import concourse.mybir as mybir
import concourse.tile as tile
from concourse import bacc
from concourse.bass_utils import run_bass_kernel_spmd
from concourse.masks import make_identity

# Problem dims (fixed by the nn_MLP_90795608637901 spec).
B, E, CAP, D = 8, 8, 1024, 768
H = 2048
H2 = 2 * H
T = B * CAP          # tokens per expert (per core) = 8192
TB = 512             # token block
NB = T // TB         # 16 blocks
S = TB // 128        # 4 partition sub-tiles per block
KC1 = D // 128       # 6 contraction chunks for GEMM1
MC = H // 128        # 16 value/gate chunk pairs
KC2 = H // 128       # 16 contraction chunks for GEMM2
NO = D // 128        # 6 output d chunks

BF = mybir.dt.bfloat16
F32 = mybir.dt.float32
I32 = mybir.dt.int32
ALU = mybir.AluOpType


def build_kernel(nb: int = NB) -> bass.Bass:
    nc = bacc.Bacc("TRN2", target_bir_lowering=False, debug=False)

    t = nb * TB
    x = nc.declare_dram_parameter("x", [t, D], BF, isOutput=False)
    w1 = nc.declare_dram_parameter("w1", [D, H2], BF, isOutput=False)
    w2 = nc.declare_dram_parameter("w2", [H, D], BF, isOutput=False)
    out = nc.declare_dram_parameter("out", [t, D], F32, isOutput=True)

    with tile.TileContext(nc) as tc, ExitStack() as ctx:
        weights = ctx.enter_context(tc.tile_pool(name="weights", bufs=1))
        io_in = ctx.enter_context(tc.tile_pool(name="io_in", bufs=3))
        io_out = ctx.enter_context(tc.tile_pool(name="io_out", bufs=2))
        work = ctx.enter_context(tc.tile_pool(name="work", bufs=2))
        gpool = ctx.enter_context(tc.tile_pool(name="gpool", bufs=1))
        small = ctx.enter_context(tc.tile_pool(name="small", bufs=2))
        agp = ctx.enter_context(tc.tile_pool(name="agp", bufs=3))
        psum_mm = ctx.enter_context(tc.tile_pool(name="psum_mm", bufs=4, space="PSUM"))
        psum_tr = ctx.enter_context(tc.tile_pool(name="psum_tr", bufs=3, space="PSUM"))

        # Resident weights: [p, k, n] with contraction index d = k*128 + p.
        w1s = weights.tile([128, KC1, H2], BF)
        for k in range(KC1):
            nc.sync.dma_start(out=w1s[:, k, :], in_=w1[k * 128:(k + 1) * 128, :])
        w2s = weights.tile([128, KC2, D], BF)
        for k in range(KC2):
            nc.sync.dma_start(out=w2s[:, k, :], in_=w2[k * 128:(k + 1) * 128, :])

        ident = weights.tile([128, 128], BF)
        make_identity(nc, ident)
        bias0 = weights.tile([128, 1], F32)
        nc.vector.memset(bias0, 0.0)

        for b in range(nb):
            xv = x[b * TB:(b + 1) * TB].rearrange("(s p) d -> p s d", p=128)
            xb = io_in.tile([128, S, D], BF, name="xb")
            nc.sync.dma_start(out=xb, in_=xv)

            # --- RMSNorm scale: y = 1/sqrt(sum(x^2)) ---
            # squared-sum on ACT (Square sits in the gelu table set), rsqrt on
            # DVE via bit-trick + Newton (keeps ACT free of table switches).
            ssb = small.tile([128, S], F32, name="ssb")
            sq = small.tile([128, D], BF, name="sq")
            for s in range(S):
                nc.scalar.activation(
                    sq, xb[:, s], mybir.ActivationFunctionType.Square,
                    bias=bias0, accum_out=ssb[:, s:s + 1],
                )
            yb = small.tile([128, S], F32, name="yb")
            tb = small.tile([128, S], F32, name="tb")
            # rsqrt seed via the int bit trick: 0x5f3759df - (i >> 1)
            # (written as (i>>1 xor -1) + 0x5f3759df + 1), then 3 Newton steps.
            nc.vector.tensor_scalar(
                out=yb.bitcast(I32), in0=ssb.bitcast(I32),
                scalar1=1, scalar2=-1,
                op0=ALU.logical_shift_right, op1=ALU.bitwise_xor,
            )
            nc.vector.tensor_scalar(
                out=yb.bitcast(I32), in0=yb.bitcast(I32),
                scalar1=0x5F375A60, scalar2=None, op0=ALU.add,
            )
            for _ in range(3):
                nc.vector.tensor_mul(tb, yb, yb)
                nc.vector.tensor_mul(tb, tb, ssb)
                nc.vector.tensor_scalar(
                    out=tb, in0=tb, scalar1=-0.5, scalar2=1.5,
                    op0=ALU.mult, op1=ALU.add,
                )
                nc.vector.tensor_mul(yb, yb, tb)

            hb = work.tile([128, S, D], BF, name="hb")
            for s in range(S):
                nc.vector.tensor_scalar_mul(hb[:, s], xb[:, s], yb[:, s:s + 1])

            # --- h^T: [d on partitions, tokens free] via PE transpose ---
            ht = work.tile([128, KC1, TB], BF, name="ht")
            for s in range(S):
                for k in range(KC1):
                    pt = psum_tr.tile([128, 128], BF, name="pt", tag="pt",
                                      space="PSUM")
                    nc.tensor.transpose(pt, hb[:, s, k * 128:(k + 1) * 128], ident)
                    nc.vector.tensor_copy(ht[:, k, s * 128:(s + 1) * 128], pt)

            # --- GEMM1 + GEGLU, one value/gate chunk pair at a time ---
            gbuf = gpool.tile([128, KC2, TB], BF, name="gbuf")
            for m in range(MC):
                pv = psum_mm.tile([128, TB], F32, name="pv", tag="mm", space="PSUM")
                pg = psum_mm.tile([128, TB], F32, name="pg", tag="mm", space="PSUM")
                for k in range(KC1):
                    nc.tensor.matmul(
                        pv, lhsT=w1s[:, k, m * 128:(m + 1) * 128], rhs=ht[:, k, :],
                        start=(k == 0), stop=(k == KC1 - 1),
                    )
                for k in range(KC1):
                    nc.tensor.matmul(
                        pg, lhsT=w1s[:, k, H + m * 128:H + (m + 1) * 128],
                        rhs=ht[:, k, :],
                        start=(k == 0), stop=(k == KC1 - 1),
                    )
                ag = agp.tile([128, TB], F32, name="ag")
                nc.scalar.activation(
                    ag, pg, mybir.ActivationFunctionType.Gelu, bias=bias0,
                )
                nc.vector.tensor_mul(gbuf[:, m, :], pv, ag)

            # --- GEMM2: out^T chunks [d on partitions, tokens free] ---
            obuf = work.tile([128, NO, TB], BF, name="obuf")
            for n in range(NO):
                po = psum_mm.tile([128, TB], F32, name="po", tag="mm", space="PSUM")
                for k2 in range(KC2):
                    nc.tensor.matmul(
                        po, lhsT=w2s[:, k2, n * 128:(n + 1) * 128],
                        rhs=gbuf[:, k2, :],
                        start=(k2 == 0), stop=(k2 == KC2 - 1),
                    )
                nc.vector.tensor_copy(obuf[:, n, :], po)

            # --- transpose back to token-major and store ---
            outb = io_out.tile([128, S, D], F32, name="outb")
            for n in range(NO):
                for s in range(S):
                    pt2 = psum_tr.tile([128, 128], BF, name="pt2", tag="pt",
                                       space="PSUM")
                    nc.tensor.transpose(pt2, obuf[:, n, s * 128:(s + 1) * 128], ident)
                    nc.vector.tensor_copy(outb[:, s, n * 128:(n + 1) * 128], pt2)
            ov = out[b * TB:(b + 1) * TB].rearrange("(s p) d -> p s d", p=128)
            nc.sync.dma_start(out=ov, in_=outb)

    nc.finalize()
    return nc


def prepare_in_maps(x, c_fc, c_proj, gamma, mult_bias):
    bf16 = ml_dtypes.bfloat16
    g = (gamma.astype(np.float32) * np.float32(np.sqrt(D)))
    w1_all = (c_fc.astype(np.float32) * g[None, :, None]).astype(bf16)
    w2_all = (c_proj.astype(np.float32)
              * mult_bias.astype(np.float32)[None, :, None]).astype(bf16)
    xs = np.ascontiguousarray(np.transpose(x, (1, 0, 2, 3))).reshape(E, T, D)
    xs = xs.astype(bf16)
    return [
        {"x": xs[e], "w1": w1_all[e], "w2": w2_all[e]}
        for e in range(E)
    ]


def run(in_maps, trace: bool = False):
    nc = build_kernel()
    return run_bass_kernel_spmd(
        nc, in_maps, core_ids=list(range(E)), trace=trace,
    )


def kernel(x, c_fc, c_proj, gamma, mult_bias):
    in_maps = prepare_in_maps(x, c_fc, c_proj, gamma, mult_bias)
    res = run(in_maps)
    out = np.empty((E, B, CAP, D), np.float32)
    for e in range(E):
        out[e] = res.results[e]["out"].reshape(B, CAP, D)
    return np.ascontiguousarray(out.transpose(1, 0, 2, 3))


# revision 8
# speedup vs baseline: 1.0925x; 1.0925x over previous
"""Expert-parallel MoE GEGLU MLP (RMSNorm -> c_fc -> GEGLU -> c_proj) on 8
Trainium2 NeuronCores.

Sharding: expert-parallel. Core e computes the full MLP for expert e's tokens
(x[:, e] -> [8192, 768]); no collectives. gamma*sqrt(D) is folded into c_fc
and mult_bias into c_proj on the host, so the device kernel computes:

    h   = x / ||x||_2            (per token, fp32 accumulate)
    u   = h @ W1                 (bf16 x bf16 -> fp32 PSUM)
    g   = gelu(u_gate) * u_val   (exact erf gelu on ACT)
    out = g @ W2                 (bf16 x bf16 -> fp32 PSUM)

Layout: tokens stream in super-blocks of 1024. x is loaded twice: once
token-major (for the squared-sum only) and once d-major via the DMA xbar
transpose straight from DRAM. The per-token rsqrt scale is computed
token-major (cheap DVE Newton), moved to a row with one tiny PE transpose,
broadcast across partitions with K=1 matmuls, and applied in place to the
transposed activations. GEMM1 runs with hidden on PSUM partitions and
1024-token moving operands; GEMM2 uses the GEGLU output chunks as the
stationary operand so its PSUM output is already token-major - no output
transposes at all.
"""

from contextlib import ExitStack

import ml_dtypes
import numpy as np

import concourse.bass as bass
import concourse.mybir as mybir
import concourse.tile as tile
from concourse import bacc
from concourse.bass_utils import run_bass_kernel_spmd
from concourse.masks import make_identity

# Problem dims (fixed by the nn_MLP_90795608637901 spec).
B, E, CAP, D = 8, 8, 1024, 768
H = 2048
H2 = 2 * H
T = B * CAP          # tokens per expert (per core) = 8192
SB = 1024            # tokens per super-block
NSB = T // SB        # 8
S = SB // 128        # 8 partition sub-tiles per super-block
KC1 = D // 128       # 6 contraction chunks for GEMM1
MC = H // 128        # 16 value/gate chunk pairs
KC2 = H // 128       # 16 contraction chunks for GEMM2

BF = mybir.dt.bfloat16
F32 = mybir.dt.float32
I32 = mybir.dt.int32
ALU = mybir.AluOpType


def build_kernel(nsb: int = NSB) -> bass.Bass:
    nc = bacc.Bacc("TRN2", target_bir_lowering=False, debug=False)

    t = nsb * SB
    x = nc.declare_dram_parameter("x", [t, D], BF, isOutput=False)
    w1 = nc.declare_dram_parameter("w1", [D, H2], BF, isOutput=False)
    w2 = nc.declare_dram_parameter("w2", [H, D], BF, isOutput=False)
    sel = nc.declare_dram_parameter("sel", [S, SB], F32, isOutput=False)
    out = nc.declare_dram_parameter("out", [t, D], F32, isOutput=True)

    with tile.TileContext(nc) as tc, ExitStack() as ctx:
        weights = ctx.enter_context(tc.tile_pool(name="weights", bufs=1))
        io_in = ctx.enter_context(tc.tile_pool(name="io_in", bufs=2))
        work = ctx.enter_context(tc.tile_pool(name="work", bufs=2))
        gpool = ctx.enter_context(tc.tile_pool(name="gpool", bufs=1))
        small = ctx.enter_context(tc.tile_pool(name="small", bufs=2))
        agp = ctx.enter_context(tc.tile_pool(name="agp", bufs=3))
        obp = ctx.enter_context(tc.tile_pool(name="obp", bufs=3))
        psum_mm = ctx.enter_context(tc.tile_pool(name="psum_mm", bufs=4, space="PSUM"))
        psum_sc = ctx.enter_context(tc.tile_pool(name="psum_sc", bufs=1, space="PSUM"))
        psum_yt = ctx.enter_context(tc.tile_pool(name="psum_yt", bufs=2, space="PSUM"))

        # x DMAs for a super-block; emitted ahead of the weight loads for
        # sb=0 so the PE pipeline can start before 19MB of weights land.
        x_tiles = {}

        def issue_x(sb):
            xb = io_in.tile([128, S, D], BF, name="xb", tag="xb")
            xv = x[sb * SB:(sb + 1) * SB].rearrange("(s p) d -> p s d", p=128)
            nc.sync.dma_start(out=xb, in_=xv)
            xt = work.tile([128, KC1, SB], BF, name="xt", tag="xt")
            for k in range(KC1):
                nc.scalar.dma_start(
                    out=xt[:, k, :],
                    in_=x[sb * SB:(sb + 1) * SB, k * 128:(k + 1) * 128],
                    transpose=True,
                )
            x_tiles[sb] = (xb, xt)

        issue_x(0)

        # Resident weights: [p, k, n] with contraction index = k*128 + p.
        w1s = weights.tile([128, KC1, H2], BF)
        for k in range(KC1):
            nc.sync.dma_start(out=w1s[:, k, :], in_=w1[k * 128:(k + 1) * 128, :])
        w2s = weights.tile([128, KC2, D], BF)
        for k in range(KC2):
            nc.sync.dma_start(out=w2s[:, k, :], in_=w2[k * 128:(k + 1) * 128, :])

        ident = weights.tile([128, 128], F32)
        make_identity(nc, ident)
        # sel[s, s*128+q] = 1: selector for the partition-broadcast matmul
        sels = weights.tile([S, SB], F32)
        nc.sync.dma_start(out=sels, in_=sel[:, :])
        bias0 = weights.tile([128, 1], F32)
        nc.vector.memset(bias0, 0.0)

        for sb in range(nsb):
            if sb + 1 < nsb:
                issue_x(sb + 1)
            xb, xt = x_tiles.pop(sb)

            # --- RMSNorm scale, token-major: ss on ACT, rsqrt on DVE ---
            ssb = small.tile([128, S], F32, name="ssb")
            sq = small.tile([128, D], BF, name="sq")
            for s in range(S):
                nc.scalar.activation(
                    sq, xb[:, s], mybir.ActivationFunctionType.Square,
                    bias=bias0, accum_out=ssb[:, s:s + 1],
                )
            yb = small.tile([128, S], F32, name="yb")
            tb = small.tile([128, S], F32, name="tb")
            # rsqrt seed via the int bit trick: 0x5f3759df - (i >> 1)
            # (written as (i>>1 xor -1) + 0x5f3759df + 1), then 3 Newton steps.
            nc.vector.tensor_scalar(
                out=yb.bitcast(I32), in0=ssb.bitcast(I32),
                scalar1=1, scalar2=-1,
                op0=ALU.logical_shift_right, op1=ALU.bitwise_xor,
            )
            nc.vector.tensor_scalar(
                out=yb.bitcast(I32), in0=yb.bitcast(I32),
                scalar1=0x5F375A60, scalar2=None, op0=ALU.add,
            )
            for _ in range(3):
                nc.vector.tensor_mul(tb, yb, yb)
                nc.vector.tensor_mul(tb, tb, ssb)
                nc.vector.tensor_scalar(
                    out=tb, in0=tb, scalar1=-0.5, scalar2=1.5,
                    op0=ALU.mult, op1=ALU.add,
                )
                nc.vector.tensor_mul(yb, yb, tb)

            # --- broadcast scale across partitions: yb[p,s] -> sc[:,s*128+p]
            yt = psum_yt.tile([S, 128], F32, name="yt", tag="yt", space="PSUM")
            nc.tensor.transpose(yt, yb, ident)
            yrow = small.tile([S, 128], F32, name="yrow")
            nc.vector.tensor_copy(yrow, yt)
            psc = psum_sc.tile([128, SB], F32, name="psc", tag="sc", space="PSUM")
            for s in range(S):
                nc.tensor.matmul(
                    psc[:, s * 128:(s + 1) * 128],
                    lhsT=sels[:, s * 128:(s + 1) * 128],
                    rhs=yrow, start=True, stop=True,
                )
            sc = work.tile([128, SB], F32, name="sc", tag="sc")
            nc.vector.tensor_copy(sc, psc)

            # --- normalize in place in the transposed domain ---
            for k in range(KC1):
                nc.vector.tensor_mul(xt[:, k, :], xt[:, k, :], sc)

            # --- GEMM1 + GEGLU, one value/gate chunk pair at a time.
            # A matmul's fp32 PSUM output cannot cross a 2KB bank, so the
            # 1024-token super-block runs as two 512-column halves. ---
            gbuf = gpool.tile([128, KC2, SB], BF, name="gbuf")
            for m in range(MC):
                for h2 in range(2):
                    cols = slice(h2 * 512, (h2 + 1) * 512)
                    pv = psum_mm.tile([128, 512], F32, name="pv", tag="mm",
                                      space="PSUM")
                    pg = psum_mm.tile([128, 512], F32, name="pg", tag="mm",
                                      space="PSUM")
                    for k in range(KC1):
                        nc.tensor.matmul(
                            pv, lhsT=w1s[:, k, m * 128:(m + 1) * 128],
                            rhs=xt[:, k, cols],
                            start=(k == 0), stop=(k == KC1 - 1),
                        )
                    for k in range(KC1):
                        nc.tensor.matmul(
                            pg, lhsT=w1s[:, k, H + m * 128:H + (m + 1) * 128],
                            rhs=xt[:, k, cols],
                            start=(k == 0), stop=(k == KC1 - 1),
                        )
                    ag = agp.tile([128, 512], F32, name="ag")
                    nc.scalar.activation(
                        ag, pg, mybir.ActivationFunctionType.Gelu, bias=bias0,
                    )
                    nc.vector.tensor_mul(gbuf[:, m, cols], pv, ag)

            # --- GEMM2 with gbuf chunks stationary: PSUM comes out
            # token-major, so results DMA straight out after one copy.
            # d=768 output splits into 512+256 PSUM chains (bank rule). ---
            for mt in range(S):
                ob = obp.tile([128, D], F32, name="ob")
                for d0, d1 in ((0, 512), (512, 768)):
                    po = psum_mm.tile([128, d1 - d0], F32, name="po", tag="mm",
                                      space="PSUM")
                    for k2 in range(KC2):
                        nc.tensor.matmul(
                            po, lhsT=gbuf[:, k2, mt * 128:(mt + 1) * 128],
                            rhs=w2s[:, k2, d0:d1],
                            start=(k2 == 0), stop=(k2 == KC2 - 1),
                        )
                    nc.vector.tensor_copy(ob[:, d0:d1], po)
                nc.sync.dma_start(
                    out=out[sb * SB + mt * 128:sb * SB + (mt + 1) * 128, :],
                    in_=ob,
                )

    nc.finalize()
    return nc


def prepare_in_maps(x, c_fc, c_proj, gamma, mult_bias):
    bf16 = ml_dtypes.bfloat16
    g = (gamma.astype(np.float32) * np.float32(np.sqrt(D)))
    w1_all = (c_fc.astype(np.float32) * g[None, :, None]).astype(bf16)
    w2_all = (c_proj.astype(np.float32)
              * mult_bias.astype(np.float32)[None, :, None]).astype(bf16)
    xs = np.ascontiguousarray(np.transpose(x, (1, 0, 2, 3))).reshape(E, T, D)
    xs = xs.astype(bf16)
    sel = np.zeros((S, SB), np.float32)
    for s in range(S):
        sel[s, s * 128:(s + 1) * 128] = 1.0
    return [
        {"x": xs[e], "w1": w1_all[e], "w2": w2_all[e], "sel": sel}
        for e in range(E)
    ]


def run(in_maps, trace: bool = False):
    nc = build_kernel()
    return run_bass_kernel_spmd(
        nc, in_maps, core_ids=list(range(E)), trace=trace,
    )


def kernel(x, c_fc, c_proj, gamma, mult_bias):
    in_maps = prepare_in_maps(x, c_fc, c_proj, gamma, mult_bias)
    res = run(in_maps)
    out = np.empty((E, B, CAP, D), np.float32)
    for e in range(E):
        out[e] = res.results[e]["out"].reshape(B, CAP, D)
    return np.ascontiguousarray(out.transpose(1, 0, 2, 3))


# revision 9
# speedup vs baseline: 1.1020x; 1.0087x over previous
"""Expert-parallel MoE GEGLU MLP (RMSNorm -> c_fc -> GEGLU -> c_proj) on 8
Trainium2 NeuronCores.

Sharding: expert-parallel. Core e computes the full MLP for expert e's tokens
(x[:, e] -> [8192, 768]); no collectives. gamma*sqrt(D) is folded into c_fc
and mult_bias into c_proj on the host, so the device kernel computes:

    h   = x / ||x||_2            (per token, fp32 accumulate)
    u   = h @ W1                 (bf16 x bf16 -> fp32 PSUM)
    g   = gelu(u_gate) * u_val   (exact erf gelu on ACT)
    out = g @ W2                 (bf16 x bf16 -> fp32 PSUM)

Layout: tokens stream in super-blocks of 1024. x is loaded twice: once
token-major (for the squared-sum only) and once d-major via the DMA xbar
transpose straight from DRAM. The per-token rsqrt scale is computed
token-major (cheap DVE Newton), moved to a row with one tiny PE transpose,
broadcast across partitions with K=1 matmuls, and applied in place to the
transposed activations. GEMM1 runs with hidden on PSUM partitions and
1024-token moving operands; GEMM2 uses the GEGLU output chunks as the
stationary operand so its PSUM output is already token-major - no output
transposes at all.
"""

from contextlib import ExitStack

import ml_dtypes
import numpy as np

import concourse.bass as bass
import concourse.mybir as mybir
import concourse.tile as tile
from concourse import bacc
from concourse.bass_utils import run_bass_kernel_spmd
from concourse.masks import make_identity

# Problem dims (fixed by the nn_MLP_90795608637901 spec).
B, E, CAP, D = 8, 8, 1024, 768
H = 2048
H2 = 2 * H
T = B * CAP          # tokens per expert (per core) = 8192
SB = 1024            # tokens per super-block
NSB = T // SB        # 8
S = SB // 128        # 8 partition sub-tiles per super-block
KC1 = D // 128       # 6 contraction chunks for GEMM1
MC = H // 128        # 16 value/gate chunk pairs
KC2 = H // 128       # 16 contraction chunks for GEMM2

BF = mybir.dt.bfloat16
F32 = mybir.dt.float32
I32 = mybir.dt.int32
ALU = mybir.AluOpType


def build_kernel(nsb: int = NSB) -> bass.Bass:
    nc = bacc.Bacc("TRN2", target_bir_lowering=False, debug=False)

    t = nsb * SB
    x = nc.declare_dram_parameter("x", [t, D], BF, isOutput=False)
    xT = nc.declare_dram_parameter("xT", [D, t], BF, isOutput=False)
    w1 = nc.declare_dram_parameter("w1", [D, H2], BF, isOutput=False)
    w2 = nc.declare_dram_parameter("w2", [H, D], BF, isOutput=False)
    sel = nc.declare_dram_parameter("sel", [S, SB], F32, isOutput=False)
    out = nc.declare_dram_parameter("out", [t, D], F32, isOutput=True)

    with tile.TileContext(nc) as tc, ExitStack() as ctx:
        weights = ctx.enter_context(tc.tile_pool(name="weights", bufs=1))
        io_in = ctx.enter_context(tc.tile_pool(name="io_in", bufs=2))
        work = ctx.enter_context(tc.tile_pool(name="work", bufs=2))
        gpool = ctx.enter_context(tc.tile_pool(name="gpool", bufs=1))
        small = ctx.enter_context(tc.tile_pool(name="small", bufs=2))
        agp = ctx.enter_context(tc.tile_pool(name="agp", bufs=3))
        obp = ctx.enter_context(tc.tile_pool(name="obp", bufs=3))
        psum_mm = ctx.enter_context(tc.tile_pool(name="psum_mm", bufs=4, space="PSUM"))
        psum_sc = ctx.enter_context(tc.tile_pool(name="psum_sc", bufs=1, space="PSUM"))
        psum_yt = ctx.enter_context(tc.tile_pool(name="psum_yt", bufs=2, space="PSUM"))

        # x DMAs for a super-block; emitted ahead of the weight loads for
        # sb=0 so the PE pipeline can start before 19MB of weights land.
        x_tiles = {}

        def issue_x(sb):
            xb = io_in.tile([128, S, D], BF, name="xb", tag="xb")
            xv = x[sb * SB:(sb + 1) * SB].rearrange("(s p) d -> p s d", p=128)
            nc.sync.dma_start(out=xb, in_=xv)
            xt = work.tile([128, KC1, SB], BF, name="xt", tag="xt")
            for k in range(KC1):
                nc.scalar.dma_start(
                    out=xt[:, k, :],
                    in_=xT[k * 128:(k + 1) * 128, sb * SB:(sb + 1) * SB],
                )
            x_tiles[sb] = (xb, xt)

        issue_x(0)

        # Resident weights: [p, k, n] with contraction index = k*128 + p.
        # W1 lands in (value-block, gate-block) column pairs so the first
        # GEMM1 chunks can start ~10us in instead of waiting for 12.6MB.
        w1s = weights.tile([128, KC1, H2], BF)
        for nb in range(4):
            for base in (0, H):
                c0, c1 = base + nb * 512, base + (nb + 1) * 512
                for k in range(KC1):
                    nc.sync.dma_start(out=w1s[:, k, c0:c1],
                                      in_=w1[k * 128:(k + 1) * 128, c0:c1])
        w2s = weights.tile([128, KC2, D], BF)
        for k in range(KC2):
            nc.sync.dma_start(out=w2s[:, k, :], in_=w2[k * 128:(k + 1) * 128, :])

        ident = weights.tile([128, 128], F32)
        make_identity(nc, ident)
        # sel[s, s*128+q] = 1: selector for the partition-broadcast matmul
        sels = weights.tile([S, SB], F32)
        nc.sync.dma_start(out=sels, in_=sel[:, :])
        bias0 = weights.tile([128, 1], F32)
        nc.vector.memset(bias0, 0.0)

        for sb in range(nsb):
            if sb + 1 < nsb:
                issue_x(sb + 1)
            xb, xt = x_tiles.pop(sb)

            # --- RMSNorm scale, token-major: ss on ACT, rsqrt on DVE ---
            ssb = small.tile([128, S], F32, name="ssb")
            sq = small.tile([128, D], BF, name="sq")
            for s in range(S):
                nc.scalar.activation(
                    sq, xb[:, s], mybir.ActivationFunctionType.Square,
                    bias=bias0, accum_out=ssb[:, s:s + 1],
                )
            yb = small.tile([128, S], F32, name="yb")
            tb = small.tile([128, S], F32, name="tb")
            # rsqrt seed via the int bit trick: 0x5f3759df - (i >> 1)
            # (written as (i>>1 xor -1) + 0x5f3759df + 1), then 3 Newton steps.
            nc.vector.tensor_scalar(
                out=yb.bitcast(I32), in0=ssb.bitcast(I32),
                scalar1=1, scalar2=-1,
                op0=ALU.logical_shift_right, op1=ALU.bitwise_xor,
            )
            nc.vector.tensor_scalar(
                out=yb.bitcast(I32), in0=yb.bitcast(I32),
                scalar1=0x5F375A60, scalar2=None, op0=ALU.add,
            )
            for _ in range(3):
                nc.vector.tensor_mul(tb, yb, yb)
                nc.vector.tensor_mul(tb, tb, ssb)
                nc.vector.tensor_scalar(
                    out=tb, in0=tb, scalar1=-0.5, scalar2=1.5,
                    op0=ALU.mult, op1=ALU.add,
                )
                nc.vector.tensor_mul(yb, yb, tb)

            # --- broadcast scale across partitions: yb[p,s] -> sc[:,s*128+p]
            yt = psum_yt.tile([S, 128], F32, name="yt", tag="yt", space="PSUM")
            nc.tensor.transpose(yt, yb, ident)
            yrow = small.tile([S, 128], F32, name="yrow")
            nc.vector.tensor_copy(yrow, yt)
            psc = psum_sc.tile([128, SB], F32, name="psc", tag="sc", space="PSUM")
            for s in range(S):
                nc.tensor.matmul(
                    psc[:, s * 128:(s + 1) * 128],
                    lhsT=sels[:, s * 128:(s + 1) * 128],
                    rhs=yrow, start=True, stop=True,
                )
            sc = work.tile([128, SB], F32, name="sc", tag="sc")
            nc.vector.tensor_copy(sc, psc)

            # --- normalize in place in the transposed domain ---
            for k in range(KC1):
                nc.vector.tensor_mul(xt[:, k, :], xt[:, k, :], sc)

            # --- GEMM1 + GEGLU, one value/gate chunk pair at a time.
            # A matmul's fp32 PSUM output cannot cross a 2KB bank, so the
            # 1024-token super-block runs as two 512-column halves. ---
            gbuf = gpool.tile([128, KC2, SB], BF, name="gbuf")
            for m in range(MC):
                for h2 in range(2):
                    cols = slice(h2 * 512, (h2 + 1) * 512)
                    pv = psum_mm.tile([128, 512], F32, name="pv", tag="mm",
                                      space="PSUM")
                    pg = psum_mm.tile([128, 512], F32, name="pg", tag="mm",
                                      space="PSUM")
                    for k in range(KC1):
                        nc.tensor.matmul(
                            pv, lhsT=w1s[:, k, m * 128:(m + 1) * 128],
                            rhs=xt[:, k, cols],
                            start=(k == 0), stop=(k == KC1 - 1),
                        )
                    for k in range(KC1):
                        nc.tensor.matmul(
                            pg, lhsT=w1s[:, k, H + m * 128:H + (m + 1) * 128],
                            rhs=xt[:, k, cols],
                            start=(k == 0), stop=(k == KC1 - 1),
                        )
                    ag = agp.tile([128, 512], F32, name="ag")
                    nc.scalar.activation(
                        ag, pg, mybir.ActivationFunctionType.Gelu, bias=bias0,
                    )
                    nc.vector.tensor_mul(gbuf[:, m, cols], pv, ag)

            # --- GEMM2 with gbuf chunks stationary: PSUM comes out
            # token-major, so results DMA straight out after one copy.
            # d=768 output splits into 512+256 PSUM chains (bank rule). ---
            for mt in range(S):
                ob = obp.tile([128, D], F32, name="ob")
                for d0, d1 in ((0, 512), (512, 768)):
                    po = psum_mm.tile([128, d1 - d0], F32, name="po", tag="mm",
                                      space="PSUM")
                    for k2 in range(KC2):
                        nc.tensor.matmul(
                            po, lhsT=gbuf[:, k2, mt * 128:(mt + 1) * 128],
                            rhs=w2s[:, k2, d0:d1],
                            start=(k2 == 0), stop=(k2 == KC2 - 1),
                        )
                    nc.vector.tensor_copy(ob[:, d0:d1], po)
                nc.sync.dma_start(
                    out=out[sb * SB + mt * 128:sb * SB + (mt + 1) * 128, :],
                    in_=ob,
                )

    nc.finalize()
    return nc


def prepare_in_maps(x, c_fc, c_proj, gamma, mult_bias):
    bf16 = ml_dtypes.bfloat16
    g = (gamma.astype(np.float32) * np.float32(np.sqrt(D)))
    w1_all = (c_fc.astype(np.float32) * g[None, :, None]).astype(bf16)
    w2_all = (c_proj.astype(np.float32)
              * mult_bias.astype(np.float32)[None, :, None]).astype(bf16)
    xs = np.ascontiguousarray(np.transpose(x, (1, 0, 2, 3))).reshape(E, T, D)
    xs = xs.astype(bf16)
    xts = np.ascontiguousarray(np.transpose(xs, (0, 2, 1)))
    sel = np.zeros((S, SB), np.float32)
    for s in range(S):
        sel[s, s * 128:(s + 1) * 128] = 1.0
    return [
        {"x": xs[e], "xT": xts[e], "w1": w1_all[e], "w2": w2_all[e], "sel": sel}
        for e in range(E)
    ]


def run(in_maps, trace: bool = False):
    nc = build_kernel()
    return run_bass_kernel_spmd(
        nc, in_maps, core_ids=list(range(E)), trace=trace,
    )


def kernel(x, c_fc, c_proj, gamma, mult_bias):
    in_maps = prepare_in_maps(x, c_fc, c_proj, gamma, mult_bias)
    res = run(in_maps)
    out = np.empty((E, B, CAP, D), np.float32)
    for e in range(E):
        out[e] = res.results[e]["out"].reshape(B, CAP, D)
    return np.ascontiguousarray(out.transpose(1, 0, 2, 3))
